# revision 3
# baseline (speedup 1.0000x reference)
"""Trainium2 Bass kernel for CustomGraphConv message passing — v3.

Same math as v2 (degree-binned identity scatter, int8 W stream) with the
per-engine layout rebalanced from the v2 trace:
  - Windows are processed in PAIRS sharing one PSUM bank [128, 512]
    (pair-tile k holds tile k of both windows side by side), so each
    matmul has a 512-wide free dim (halves PE instruction count) and
    PSUM eviction is ONE merged tensor_reduce per pair (DVE small ops
    were 10x over model in v2).
  - bias enters via one extra constant matmul per pair (PE is cheap).
  - relu runs on ScalarE (activation), out-DMA issued from the scalar
    queue right after it (same engine -> no cross-queue wait).
  - mult split: frac_a (default .85) int8->bf16 convert on ScalarE
    (measured 2x = 307G/s) + DVE bf16 TT mult (2x); frac_c (default .15)
    GpSimd int8 TT mult.  No DVE int8 path (1x, not worth it).
  - W8 + xj DMA both on sync queue; pairs are streamed in chunks of <=18
    pair-tiles for SBUF headroom and finer pipelining.
"""

import os
import sys
import numpy as np

sys.path.insert(0, "/opt/trn_rl_repo")

_LAST_RUN_INFO = {}

N_CORES = 8
BLK = 128
IN_C = 16
OUT_C = 16
CH = 18              # pair-tiles per processing chunk


def _install_ntff_hook():
    import types
    import contextlib
    import ctypes

    if "antenv.axon_hooks" in sys.modules:
        return
    try:
        import antenv.axon_hooks  # noqa: F401
        return
    except ImportError:
        pass

    mod = types.ModuleType("antenv.axon_hooks")
    mod._hook = None
    mod._tried = False

    def set_axon_ntff_profile_hook(h):
        mod._hook = h

    def _via_ctypes(so_path):
        lib = ctypes.CDLL(so_path)
        if not hasattr(lib, "axon_start_nrt_profile"):
            return None
        lib.axon_start_nrt_profile.argtypes = [
            ctypes.POINTER(ctypes.c_int64),
            ctypes.c_size_t,
        ]
        lib.axon_start_nrt_profile.restype = ctypes.c_int64
        lib.axon_stop_nrt_profile.argtypes = [ctypes.c_char_p]
        lib.axon_stop_nrt_profile.restype = ctypes.c_int64

        @contextlib.contextmanager
        def _hook_cm(output_dir, device_ids):
            import jax

            jax.devices()
            if device_ids:
                ids = (ctypes.c_int64 * len(device_ids))(*device_ids)
                rc = lib.axon_start_nrt_profile(ids, len(device_ids))
            else:
                rc = lib.axon_start_nrt_profile(None, 0)
            if rc != 0:
                raise RuntimeError(f"axon_start_nrt_profile rc={rc}")
            try:
                yield
            finally:
                n = lib.axon_stop_nrt_profile(str(output_dir).encode())
                print(f"profile: {n} file(s) written to {output_dir}",
                      file=sys.stderr)

        return _hook_cm

    def get_axon_ntff_profile_hook():
        if mod._hook is None and not mod._tried:
            mod._tried = True
            so = os.environ.get("AXON_PJRT_SO", "/opt/axon/libaxon_pjrt.so")
            if os.path.exists(so):
                try:
                    mod._hook = _via_ctypes(so)
                except OSError:
                    mod._hook = None
        return mod._hook

    mod.set_axon_ntff_profile_hook = set_axon_ntff_profile_hook
    mod.get_axon_ntff_profile_hook = get_axon_ntff_profile_hook
    sys.modules["antenv.axon_hooks"] = mod


def _chunk_plan(env2, f1, fc):
    """Per chunk: (pair, c0, cw, t1, t2, tc, o1, o2) with running offsets
    into the bf16 and int8 W streams (in pair-column units of 256)."""
    plan = []
    o1 = o2 = 0
    for j, T in enumerate(env2):
        c0 = 0
        while c0 < T:
            cw = min(CH, T - c0)
            t1 = int(round(f1 * cw))
            tc = int(round(fc * cw))
            if t1 + tc > cw:
                tc = cw - t1
            t2 = cw - t1 - tc
            plan.append((j, c0, cw, t1, t2, tc, o1, o2))
            o1 += 2 * t1
            o2 += 2 * (t2 + tc)
            c0 += cw
    return plan, o1, o2


def _build_bass(env2, plan, b1, b2):
    import concourse.bacc as bacc
    import concourse.tile as tile
    import concourse.mybir as mybir

    f32 = mybir.dt.float32
    f16 = mybir.dt.bfloat16
    i8 = mybir.dt.int8
    npair = len(env2)
    off = [0]
    for t in env2:
        off.append(off[-1] + t)
    tiles2 = off[-1]

    nc = bacc.Bacc("TRN2", target_bir_lowering=False, debug=False,
                   num_devices=N_CORES)

    wb_d = nc.dram_tensor("wbf", [128, max(b1, 1), 256], f16,
                          kind="ExternalInput")
    w8_d = nc.dram_tensor("w8", [128, max(b2, 1), 256], i8,
                          kind="ExternalInput")
    xj_d = nc.dram_tensor("xj", [128, tiles2, 32], f16, kind="ExternalInput")
    id_d = nc.dram_tensor("ident", [128, BLK], f16, kind="ExternalInput")
    bq_d = nc.dram_tensor("biasq", [128, 512], f16, kind="ExternalInput")
    out_d = nc.dram_tensor("out", [128, 2 * npair, OUT_C], f32,
                           kind="ExternalOutput")

    with tile.TileContext(nc) as tc:
        with (
            tc.tile_pool(name="wpool", bufs=3) as wpool,
            tc.tile_pool(name="wbpool", bufs=3) as wbpool,
            tc.tile_pool(name="xpool", bufs=4) as xpool,
            tc.tile_pool(name="cpool", bufs=1) as cpool,
            tc.tile_pool(name="wcpool", bufs=2) as wcpool,
            tc.tile_pool(name="qpool", bufs=3) as qpool,
            tc.tile_pool(name="opool", bufs=4) as opool,
            tc.tile_pool(name="psum", bufs=4, space="PSUM") as psum_pool,
        ):
            ident_t = cpool.tile([128, BLK], f16, tag="ident")
            nc.sync.dma_start(ident_t[:], id_d[:])
            biasq_t = cpool.tile([128, 512], f16, tag="biasq")
            nc.sync.dma_start(biasq_t[:], bq_d[:])

            ps_tiles = {}

            def emit_chunk(j, c0, cw, t1, t2, tc_, o1, o2):
                T = env2[j]
                base = off[j]
                if c0 == 0:
                    ps = psum_pool.tile([128, 512], f32)
                    ps_tiles[j] = ps
                    nc.tensor.matmul(ps[:], ident_t[:], biasq_t[:],
                                     start=True, stop=False)
                ps = ps_tiles[j]
                xt = xpool.tile([128, CH, 32], f16, tag="xt")
                nc.sync.dma_start(xt[:, :cw, :],
                                  xj_d[:, base + c0:base + c0 + cw, :])
                qt = qpool.tile([128, CH, 2, OUT_C, IN_C], f16, tag="qt")
                qf = qt.rearrange("p g t o i -> p (g t) o i")
                xf = xt[:, :cw, :].rearrange(
                    "p g (t i) -> p (g t) i", i=IN_C)

                def mult(engine, lo, hi, src):
                    engine.tensor_tensor(
                        qf[:, 2 * lo:2 * hi, :, :],
                        src.rearrange("p G (o i) -> p G o i", i=IN_C),
                        xf[:, 2 * lo:2 * hi, :].unsqueeze(2)
                            .broadcast_to(
                                [128, 2 * (hi - lo), OUT_C, IN_C]),
                        op=mybir.AluOpType.mult,
                    )

                if t1 > 0:
                    wb = wbpool.tile([128, 2 * CH, 256], f16, tag="wb")
                    nc.sync.dma_start(wb[:, :2 * t1, :],
                                      wb_d[:, o1:o1 + 2 * t1, :])
                    mult(nc.vector, 0, t1, wb[:, :2 * t1, :])
                n8 = 2 * (t2 + tc_)
                if n8 > 0:
                    wt = wpool.tile([128, 2 * CH, 256], i8, tag="wt")
                    nc.sync.dma_start(wt[:, :n8, :], w8_d[:, o2:o2 + n8, :])
                if t2 > 0:
                    wc = wcpool.tile([128, 2 * CH, 256], f16, tag="wc")
                    nc.scalar.copy(wc[:, :2 * t2, :], wt[:, :2 * t2, :])
                    mult(nc.vector, t1, t1 + t2, wc[:, :2 * t2, :])
                if tc_ > 0:
                    mult(nc.gpsimd, t1 + t2, cw,
                         wt[:, 2 * t2:n8, :])
                for k in range(cw):
                    nc.tensor.matmul(
                        ps[:],
                        ident_t[:],
                        qt[:, k, :, :, :],
                        start=False,
                        stop=(c0 + k == T - 1),
                    )

            def emit_back(j):
                ps = ps_tiles.pop(j)
                ot = opool.tile([128, 2, OUT_C], f32, tag="ot")
                nc.vector.tensor_reduce(
                    ot[:],
                    ps[:].rearrange("p (w o i) -> p w o i", o=OUT_C, i=IN_C),
                    axis=mybir.AxisListType.X,
                    op=mybir.AluOpType.add,
                )
                orl = opool.tile([128, 2, OUT_C], f32, tag="orl")
                nc.scalar.activation(
                    orl[:], ot[:], mybir.ActivationFunctionType.Relu)
                nc.scalar.dma_start(out_d[:, 2 * j:2 * j + 2, :], orl[:])

            done = -1
            for (j, c0, cw, t1, t2, tc_, o1, o2) in plan:
                if j >= 2 and done < j - 2:
                    emit_back(j - 2)
                    done = j - 2
                emit_chunk(j, c0, cw, t1, t2, tc_, o1, o2)
            for j in range(max(done + 1, 0), npair):
                emit_back(j)

    nc.compile()
    return nc


def kernel(x, edge_index, edge_attr, weights_matrices, bias,
           input_size, output_size, **_unused):
    _install_ntff_hook()
    import ml_dtypes

    f1 = float(os.environ.get("GNN_F1", "0.35"))
    fc = float(os.environ.get("GNN_FC", "0.08"))

    x = np.asarray(x, dtype=np.float32)
    edge_index = np.asarray(edge_index)
    W = np.asarray(weights_matrices, dtype=np.float32)
    bias = np.asarray(bias, dtype=np.float32)

    N = x.shape[0]
    E = edge_index.shape[1]
    npc = (N + N_CORES - 1) // N_CORES
    nblk = (npc + BLK - 1) // BLK
    if nblk % 2:
        nblk += 1                                      # whole pairs
    npair = nblk // 2

    src = edge_index[0].astype(np.int64)
    dst = edge_index[1].astype(np.int64)
    core = dst // npc
    dl = dst - core * npc

    s_w = float(np.abs(W).max()) / 127.0
    W8 = np.clip(np.rint(W.reshape(E, 256) / s_w), -127, 127).astype(np.int8)

    deg = np.bincount(core * npc + dl, minlength=N_CORES * npc) \
        .reshape(N_CORES, npc)
    node_order = np.argsort(-deg, axis=1, kind="stable")
    rank = np.empty_like(node_order)
    rank[np.arange(N_CORES)[:, None], node_order] = np.arange(npc)[None, :]

    degs = np.take_along_axis(deg, node_order, axis=1)
    pad_deg = np.zeros((N_CORES, nblk * BLK), np.int64)
    pad_deg[:, :npc] = degs
    t_cb = pad_deg.reshape(N_CORES, nblk, BLK).max(axis=2)
    env = np.maximum(t_cb.max(axis=0), 1).astype(np.int64)
    env2 = env.reshape(npair, 2).max(axis=1)           # pair tile counts
    off2 = np.zeros(npair + 1, np.int64)
    np.cumsum(env2, out=off2[1:])
    tiles2 = int(off2[-1])
    epc = tiles2 * 2 * BLK

    key = core * npc + dl
    order_e = np.argsort(key, kind="stable")
    key_s = key[order_e]
    gstart = np.zeros(N_CORES * npc + 1, np.int64)
    np.cumsum(np.bincount(key_s, minlength=N_CORES * npc), out=gstart[1:])
    t_e = np.arange(E, dtype=np.int64) - gstart[key_s]

    core_s = key_s // npc
    rank_s = rank[core_s, key_s - core_s * npc]
    blk_s = rank_s // BLK
    part_s = rank_s - blk_s * BLK
    pj = blk_s // 2
    side = blk_s - 2 * pj
    slot = ((off2[pj] + t_e) * 2 + side) * BLK + part_s

    perm = np.full((N_CORES, epc), -1, dtype=np.int64)
    perm[core_s, slot] = order_e
    pad_mask = perm < 0
    perm_c = np.where(pad_mask, 0, perm)

    def to_tiles(a):
        F = a.shape[-1]
        return np.ascontiguousarray(
            a.reshape(N_CORES, tiles2 * 2, BLK, F).transpose(0, 2, 1, 3))

    def chunk_plan_cols(env2_l, off2_a):
        plan_l, cbf, c8 = [], [], []
        o1 = o2 = 0
        for j, T in enumerate(env2_l):
            c0 = 0
            while c0 < T:
                cw = min(CH, T - c0)
                t1 = int(round(f1 * cw))
                tcg = int(round(fc * cw))
                if t1 + tcg > cw:
                    tcg = cw - t1
                t2 = cw - t1 - tcg
                plan_l.append((j, c0, cw, t1, t2, tcg, o1, o2))
                g0 = int(off2_a[j]) + c0
                cbf.extend(range(2 * g0, 2 * (g0 + t1)))
                c8.extend(range(2 * (g0 + t1), 2 * (g0 + cw)))
                o1 += 2 * t1
                o2 += 2 * (t2 + tcg)
                c0 += cw
        return (plan_l, o1, o2, np.asarray(cbf, np.int64),
                np.asarray(c8, np.int64))

    plan, b1, b2, cols_bf, cols_8 = chunk_plan_cols(
        [int(t) for t in env2], off2)

    perm_cols = perm_c.reshape(N_CORES, tiles2 * 2, BLK)
    padm_cols = pad_mask.reshape(N_CORES, tiles2 * 2, BLK)

    Wb16 = W.reshape(E, 256).astype(ml_dtypes.bfloat16)
    wbf = Wb16[perm_cols[:, cols_bf, :]]
    wbf[padm_cols[:, cols_bf, :]] = 0
    wbf = np.ascontiguousarray(wbf.transpose(0, 2, 1, 3))

    w8p = W8[perm_cols[:, cols_8, :]]
    w8p[padm_cols[:, cols_8, :]] = 0
    w8p = np.ascontiguousarray(w8p.transpose(0, 2, 1, 3))

    # bf16-direct tiles bypass the int8 scale; int8 tiles need it folded
    # into xj.  Use two xj variants?  No: fold s_w into the int8 VALUES'
    # consumer instead — xj carries s_w for all paths, so pre-divide the
    # bf16 W tiles by s_w to compensate.
    wbf = (wbf.astype(np.float32) / s_w).astype(ml_dtypes.bfloat16)

    xs = (x * s_w).astype(ml_dtypes.bfloat16)
    xj = xs[src[perm_c]]
    xj = to_tiles(xj).reshape(N_CORES, 128, tiles2, 32)

    ident = np.eye(BLK, dtype=ml_dtypes.bfloat16)
    biasq = np.zeros((128, 2, OUT_C, IN_C), np.float32)
    biasq[:, :, :, 0] = bias[None, None, :]
    biasq = biasq.reshape(128, 512).astype(ml_dtypes.bfloat16)

    from concourse.bass_utils import run_bass_kernel_spmd

    nc = _build_bass([int(t) for t in env2], plan, b1, b2)

    in_maps = [
        {
            "wbf": np.ascontiguousarray(wbf[c]) if b1 else
                   np.zeros((128, 1, 256), ml_dtypes.bfloat16),
            "w8": np.ascontiguousarray(w8p[c]) if b2 else
                  np.zeros((128, 1, 256), np.int8),
            "xj": np.ascontiguousarray(xj[c]),
            "ident": ident,
            "biasq": biasq,
        }
        for c in range(N_CORES)
    ]

    trace = bool(int(os.environ.get("GNN_TRACE", "0")))
    res = run_bass_kernel_spmd(
        nc, in_maps, core_ids=list(range(N_CORES)), trace=trace)

    _LAST_RUN_INFO.clear()
    _LAST_RUN_INFO.update(
        exec_time_ns=res.exec_time_ns,
        mean_exec_time_ns=res.mean_exec_time_ns,
        tiles_total=tiles2,
        t_per_blk=float(np.mean(env2)),
        profile_json=res.profile_json,
        instructions_and_trace=res.instructions_and_trace,
    )

    outs = []
    for c in range(N_CORES):
        flat = res.results[c]["out"].transpose(1, 0, 2).reshape(
            nblk * BLK, OUT_C)                          # rank-major rows
        oc = np.empty((npc, OUT_C), np.float32)
        oc[node_order[c]] = flat[:npc]
        outs.append(oc)
    out = np.concatenate(outs, axis=0)
    return out[:N]
